# revision 1
# baseline (speedup 1.0000x reference)
"""Trainium2 Bass kernel: per-(image, channel) class-mean replacement (segment mean + gather).

Input:  img [8, 128, 256, 256] f32, gt [8, 1, 256, 256] int32 (labels in [0, 21))
Output: out[b, c, h, w] = mean over pixels p of img[b, c, p] where gt[b, p] == gt[b, h, w]

Sharding: data-parallel over batch — each of the 8 NeuronCores processes one image.

Per-core algorithm (C=128 channels on partitions, HW=65536 pixels on free axis):
  Phase 1 (sums):   PE-transpose 128x128 img chunks to [pix, ch]; build one-hot
                    [128pix, 32cls] (bf16) from gt via per-partition is_equal against
                    an iota row; matmul-accumulate sums[32, 129] in PSUM (col 128
                    multiplies a ones column -> per-class pixel counts).
  Means:            means[c, ch] = sums * reciprocal(counts + eps)  (bf16, stationary).
  Phase 2 (gather): PE-transpose the stashed one-hots to [32cls, 128pix]; matmul
                    means.T @ onehot -> out[128ch, pix] in PSUM; copy to SBUF; DMA out.
"""

import os
import sys

for _p in ("/opt/trn_rl_repo", "/root/.axon_site/_ro/trn_rl_repo"):
    if os.path.isdir(_p) and _p not in sys.path:
        sys.path.append(_p)

import numpy as np

P = 128          # channels == SBUF partitions
HW = 256 * 256   # pixels per image
NCLS = 21
CPAD = 32        # padded class count (transpose-block friendly)
CH = 128         # pixels per matmul chunk
NCH = HW // CH   # 512 chunks
FB = 2048        # pixels per DMA tile
NB = HW // FB    # 32 big tiles
CPB = FB // CH   # 16 chunks per big tile
EPS = 1e-8
N_CORES = 8

_CACHE = {}


def _build_module(variant="full"):
    import concourse.bacc as bacc
    import concourse.mybir as mybir
    import concourse.tile as tile
    from concourse.masks import make_identity

    do_p1 = variant in ("full", "p1")
    do_p2 = variant in ("full", "p2")
    dma_only = variant == "dma"

    f32 = mybir.dt.float32
    bf16 = mybir.dt.bfloat16
    i32 = mybir.dt.int32
    EQ = mybir.AluOpType.is_equal
    MULT = mybir.AluOpType.mult

    nc = bacc.Bacc("TRN2", target_bir_lowering=False, debug=False)
    img = nc.dram_tensor("img", [P, HW], f32, kind="ExternalInput")
    gt = nc.dram_tensor("gt", [HW], i32, kind="ExternalInput")
    out = nc.dram_tensor("out", [P, HW], f32, kind="ExternalOutput")

    with tile.TileContext(nc) as tc:
        with (
            tc.tile_pool(name="constp", bufs=1) as constp,
            tc.tile_pool(name="imgp", bufs=4) as imgp,
            tc.tile_pool(name="rhsp", bufs=6) as rhsp,
            tc.tile_pool(name="stashp", bufs=1) as stashp,
            tc.tile_pool(name="ohsbp", bufs=6) as ohsbp,
            tc.tile_pool(name="outp", bufs=6) as outp,
            tc.tile_pool(name="psA", bufs=4, space="PSUM") as psA,
            tc.tile_pool(name="psB", bufs=1, space="PSUM") as psB,
            tc.tile_pool(name="psC", bufs=2, space="PSUM") as psC,
        ):
            # ---- constants ----
            ident32 = constp.tile([P, P], f32, tag="id32")
            make_identity(nc, ident32[:])
            ident16 = constp.tile([P, P], bf16, tag="id16")
            nc.vector.tensor_copy(out=ident16[:], in_=ident32[:])
            iota = constp.tile([P, CPAD], f32, tag="iota")
            for c in range(CPAD):
                nc.vector.memset(iota[:, c : c + 1], float(c))

            # gt transposed to [128 pix, 512 chunk] so each chunk's labels sit on
            # partitions. Load gt naturally [32, 2048] (contiguous rows), cast to
            # f32, then PE-transpose 16 blocks of [32, 128] -> [128, 32]; block b
            # col r is chunk r*16+b, so copies write gtT with a stride-16 col AP.
            gtn_i = constp.tile([32, HW // 32], i32, tag="gtn_i")
            nc.sync.dma_start(
                out=gtn_i[:], in_=gt.ap().rearrange("(p f) -> p f", p=32)
            )
            gtn = constp.tile([32, HW // 32], f32, tag="gtn")
            nc.vector.tensor_copy(out=gtn[:], in_=gtn_i[:])
            gtT = constp.tile([P, NCH], f32, tag="gtT")
            for b in range(16):
                gps = psC.tile([P, 32], f32, tag="c")
                nc.tensor.transpose(
                    out=gps[:],
                    in_=gtn[:, b * P : (b + 1) * P],
                    identity=ident32[0:32, 0:32],
                )
                nc.vector.tensor_copy(out=gtT[:, b * 32 : (b + 1) * 32], in_=gps[:])

            def gtcol(gc):
                # chunk gc lives at block b=gc%16, row r=gc//16 -> col 32b+r
                return 32 * (gc % 16) + gc // 16

            # one-hot stash for the whole image: chunk gc occupies cols [32gc, 32gc+32)
            stash = stashp.tile([P, CPAD * NCH], bf16, tag="stash")

            sums = psB.tile([CPAD, P], f32, tag="sums")
            cntp = psB.tile([CPAD, 4], f32, tag="cntp")
            ones1 = constp.tile([P, 1], bf16, tag="ones1")
            nc.vector.memset(ones1[:], 1.0)

            # ---- phase 1: per-class sums + counts ----
            for t in range(NB):
                ib = imgp.tile([P, FB], f32, tag="img")
                # per-512px in-DMAs: shorter dependency tails into the sums
                # pipeline (the means barrier waits on the last one)
                for jj in range(4):
                    nc.sync.dma_start(
                        out=ib[:, jj * 512 : (jj + 1) * 512],
                        in_=img.ap()[:, t * FB + jj * 512 : t * FB + (jj + 1) * 512],
                    )
                if dma_only or variant == "p1":
                    nc.sync.dma_start(
                        out=out.ap()[:, t * FB : (t + 1) * FB], in_=ib[:]
                    )
                if dma_only:
                    continue
                for jj in range(4):
                    g4 = t * 4 + jj
                    tp4 = psA.tile([P, 512], f32, tag="a")
                    rhs4 = rhsp.tile([P, 512], bf16, tag="rhs")
                    for q in range(4):
                        gc = g4 * 4 + q
                        nc.tensor.transpose(
                            out=tp4[:, q * CH : (q + 1) * CH],
                            in_=ib[:, (jj * 4 + q) * CH : (jj * 4 + q + 1) * CH],
                            identity=ident32[:],
                        )
                        oh = stash[:, gc * CPAD : (gc + 1) * CPAD]
                        nc.vector.tensor_scalar(
                            oh, iota[:], gtT[:, gtcol(gc) : gtcol(gc) + 1], None, EQ
                        )
                    if g4 % 2 == 0:
                        nc.scalar.copy(out=rhs4[:], in_=tp4[:])
                    else:
                        nc.vector.tensor_copy(out=rhs4[:], in_=tp4[:])
                    for q in range(4):
                        gc = g4 * 4 + q
                        oh = stash[:, gc * CPAD : (gc + 1) * CPAD]
                        nc.tensor.matmul(
                            out=sums[:],
                            lhsT=oh,
                            rhs=rhs4[:, q * CH : (q + 1) * CH],
                            start=(gc == 0),
                            stop=(gc == NCH - 1),
                        )
                        nc.tensor.matmul(
                            out=cntp[:, 0:1],
                            lhsT=oh,
                            rhs=ones1[:],
                            start=(gc == 0),
                            stop=(gc == NCH - 1),
                        )

            # ---- means ----
            if do_p2:
                cnt = constp.tile([CPAD, 1], f32, tag="cnt")
                nc.vector.tensor_scalar_add(cnt[:], cntp[:, 0:1], EPS)
                rcp = constp.tile([CPAD, 1], f32, tag="rcp")
                nc.vector.reciprocal(out=rcp[:], in_=cnt[:])
                means = constp.tile([CPAD, P], bf16, tag="means")
                nc.vector.tensor_scalar(means[:], sums[:], rcp[:, 0:1], None, MULT)

            # ---- phase 2: gather out[ch, p] = means[gt[p], ch] ----
            for t in range(NB if do_p2 else 0):
                for j in range(4):
                    g = t * 4 + j
                    ob = outp.tile([P, 512], f32, tag="ob")
                    op_ = psA.tile([P, 512], f32, tag="a")
                    ohp4 = psC.tile([CPAD, 512], bf16, tag="c")
                    for q in range(4):
                        gc = g * 4 + q
                        nc.tensor.transpose(
                            out=ohp4[:, q * CH : (q + 1) * CH],
                            in_=stash[:, gc * CPAD : (gc + 1) * CPAD],
                            identity=ident16[:],
                        )
                    ohs = ohsbp.tile([CPAD, 512], bf16, tag="oh")
                    if g % 2 == 0:
                        nc.vector.tensor_copy(out=ohs[:], in_=ohp4[:])
                    else:
                        nc.scalar.copy(out=ohs[:], in_=ohp4[:])
                    nc.tensor.matmul(
                        out=op_[:], lhsT=means[:], rhs=ohs[:], start=True, stop=True
                    )
                    if g % 2 == 0:
                        nc.scalar.copy(out=ob[:], in_=op_[:])
                    else:
                        nc.vector.tensor_copy(out=ob[:], in_=op_[:])
                    nc.sync.dma_start(
                        out=out.ap()[:, g * 512 : (g + 1) * 512], in_=ob[:]
                    )

    nc.compile()
    return nc


def get_module():
    if "nc" not in _CACHE:
        _CACHE["nc"] = _build_module()
    return _CACHE["nc"]


def kernel(img, gt):
    from concourse.bass_utils import run_bass_kernel_spmd

    img = np.asarray(img)
    gt = np.asarray(gt)
    B, C, H, W = img.shape
    assert (B, C, H * W) == (N_CORES, P, HW), (img.shape,)
    img2 = np.ascontiguousarray(img.reshape(B, C, H * W))
    gt2 = np.ascontiguousarray(gt.reshape(B, H * W))

    nc = get_module()
    in_maps = [{"img": img2[i], "gt": gt2[i]} for i in range(B)]
    res = run_bass_kernel_spmd(nc, in_maps, core_ids=list(range(N_CORES)))
    out = np.stack([res.results[i]["out"] for i in range(B)], axis=0)
    return out.reshape(B, C, H, W).astype(np.float32, copy=False)


if __name__ == "__main__":
    # quick self-exercise with random data
    rng = np.random.default_rng(0)
    img = rng.standard_normal((8, 128, 256, 256), dtype=np.float32)
    gt = rng.integers(0, NCLS, size=(8, 1, 256, 256), dtype=np.int32)
    out = kernel(img=img, gt=gt)
    print("out", out.shape, out.dtype)

